# revision 11
# baseline (speedup 1.0000x reference)
"""GQA attention (B=2,S=2048,DIM=2048,H=32,KVH=8,HD=64) + RoPE, causal.

Distributed over 8 TRN2 NeuronCores: core = 4*batch + head_group.
Each core computes attention for its 8 q-heads (2 kv-heads) of one batch.

v2 design (single interleaved per-chunk pipeline):
  - x is staged f32->bf16 by one casting DRAM->DRAM DMA per chunk (gpsimd
    SWDGE), then xbar-transposed into SBUF across both HWDGE rings.
  - Q^T/K^T are produced directly by the PE (weights stationary, x^T
    moving), so no DRAM round-trip for roped q/k.  RoPE runs in the
    transposed [hd, seq] layout: a constant 128x128 block-swap matmul
    provides the rotated halves, then 3 DVE ops combine with cos/sin
    tables (host-prepared constant layout).
  - V is computed naturally (x^T stationary, wv moving) for the PV matmul.
  - Attention per chunk interleaves with the next chunk's projections and
    the previous chunk's output projection in one PE stream.
  - AllGather runs per chunk (4 small collectives), each core then
    output-projects 128 rows per chunk against the full permuted wo.
"""
import numpy as np

import concourse.bass as bass
import concourse.bacc as bacc
import concourse.tile as tile
from concourse.tile import add_dep_helper
import concourse.mybir as mybir
from concourse import bass_utils


def _ensure_axon_hooks_shim():
    """bass_utils imports antenv.axon_hooks when BASS_TRACE is set; the
    module is absent in some images. Provide a no-op shim so tracing env
    vars cannot crash the run."""
    import sys, types
    try:
        import antenv  # noqa
        if "antenv.axon_hooks" in sys.modules:
            return
        import importlib
        try:
            importlib.import_module("antenv.axon_hooks")
            return
        except ImportError:
            pass
        mod = types.ModuleType("antenv.axon_hooks")
        mod._hook = None
        mod.get_axon_ntff_profile_hook = lambda: mod._hook

        def set_axon_ntff_profile_hook(h):
            mod._hook = h
        mod.set_axon_ntff_profile_hook = set_axon_ntff_profile_hook
        sys.modules["antenv.axon_hooks"] = mod
        antenv.axon_hooks = mod
    except Exception:
        pass


_ensure_axon_hooks_shim()

F32 = mybir.dt.float32
BF16 = mybir.dt.bfloat16

B, S, DIM = 2, 2048, 2048
H, KVH, HD = 32, 8, 64
N_CORES = 8
GROUPS = [[0, 1, 2, 3], [4, 5, 6, 7]]
NCH = 4            # sequence chunks of 512
CHUNK = S // NCH   # 512
DT = DIM // 128    # 16 contraction tiles
# q-head slot order: slot s holds local q-head s//2 + 4*(s%2); slot parity
# is the local kv-head. slot-pair sp covers slots (2sp, 2sp+1).
SLOT_TO_LOCAL = [s // 2 + 4 * (s % 2) for s in range(8)]
# rope pair permutation within one head: evens then odds
HD_PERM = np.concatenate([np.arange(0, HD, 2), np.arange(1, HD, 2)])


def _build():
    nc = bacc.Bacc("TRN2", target_bir_lowering=False, debug=False,
                   num_devices=N_CORES)
    x_d = nc.dram_tensor("x", [S, DIM], F32, kind="ExternalInput")
    wq_d = nc.dram_tensor("wq", [DIM, 512], F32, kind="ExternalInput")
    wk_d = nc.dram_tensor("wk", [DIM, 128], F32, kind="ExternalInput")
    wv_d = nc.dram_tensor("wv", [DIM, 128], F32, kind="ExternalInput")
    wo_d = nc.dram_tensor("wo", [DIM, DIM], F32, kind="ExternalInput")
    cst_d = nc.dram_tensor("cst", [128, S], F32, kind="ExternalInput")
    snt_d = nc.dram_tensor("snt", [128, S], F32, kind="ExternalInput")
    psw_d = nc.dram_tensor("psw", [128, 128], F32, kind="ExternalInput")
    psel_d = nc.dram_tensor("psel", [128, 64], F32, kind="ExternalInput")
    tmsk_d = nc.dram_tensor("tmsk", [128, 128], F32, kind="ExternalInput")
    goffc_d = nc.dram_tensor("goffc", [1, 1], mybir.dt.uint32,
                             kind="ExternalInput")
    out_d = nc.dram_tensor("out", [CHUNK, DIM], F32, kind="ExternalOutput")

    Exp = mybir.ActivationFunctionType.Exp

    with tile.TileContext(nc) as tc:
        with tc.tile_pool(name="dram", bufs=1, space="DRAM") as dram, \
             tc.tile_pool(name="wpool", bufs=1) as wpool, \
             tc.tile_pool(name="io", bufs=2) as io, \
             tc.tile_pool(name="accps", bufs=2, space="PSUM") as accps, \
             tc.tile_pool(name="spsps", bufs=2, space="PSUM") as spsps, \
             tc.tile_pool(name="apsps", bufs=2, space="PSUM") as apsps:
            # ---- DRAM scratch ----
            xbf = dram.tile([S, DIM], BF16)
            agin = [dram.tile([512, CHUNK], BF16, name=f"agin{c}")
                    for c in range(NCH)]
            gath = [dram.tile([4 * 512, CHUNK], BF16, name=f"gath{c}")
                    for c in range(NCH)]

            # ---- persistent SBUF ----
            wq_sb = wpool.tile([128, DT, 512], BF16)
            wk_sb = wpool.tile([128, DT, 128], BF16)
            wv_sb = wpool.tile([128, DT, 128], BF16)
            wo_sb = wpool.tile([128, DT, DIM], BF16)
            cst_sb = wpool.tile([128, S], BF16)
            snt_sb = wpool.tile([128, S], BF16)
            psw_sb = wpool.tile([128, 128], BF16)
            psel_sb = wpool.tile([128, 64], BF16)
            tmsk_sb = wpool.tile([128, 128], BF16)
            kt_sb = wpool.tile([128, S], BF16)          # K^T (kv0|kv1)
            v_sb = wpool.tile([128, DT, 130], BF16)     # [V0|1|V1|1] per tile
            e_sb = wpool.tile([8, 512], BF16)           # recip expand indic

            # ---- prologue: stage x chunk 0 first ----
            def stage_x(c):
                # HWDGE f32 load -> DVE cast -> HWDGE store to xbf; keeps
                # the big x traffic off the single SWDGE queue
                for t in range(4):
                    r0 = c * CHUNK + t * 128
                    for hh in range(2):
                        c0 = hh * 1024
                        xf = io.tile([128, 1024], F32, tag="xf", bufs=4,
                                     name="xf")
                        nc.scalar.dma_start(xf[:],
                                            x_d[r0:r0 + 128, c0:c0 + 1024])
                        xb = io.tile([128, 1024], BF16, tag="xb", bufs=2,
                                     name="xb")
                        nc.vector.tensor_copy(xb[:], xf[:])
                        nc.sync.dma_start(xbf[r0:r0 + 128, c0:c0 + 1024],
                                          xb[:])

            stage_x(0)

            # weights + constants (casting SWDGE loads), ordered by need
            nc.gpsimd.dma_start(
                wq_sb[:], wq_d[:].rearrange("(t p) c -> p t c", p=128))
            nc.gpsimd.dma_start(
                wk_sb[:], wk_d[:].rearrange("(t p) c -> p t c", p=128))
            nc.gpsimd.dma_start(
                wv_sb[:], wv_d[:].rearrange("(t p) c -> p t c", p=128))
            nc.gpsimd.dma_start(cst_sb[:], cst_d[:])
            nc.gpsimd.dma_start(snt_sb[:], snt_d[:])
            nc.gpsimd.dma_start(psw_sb[:], psw_d[:])
            nc.gpsimd.dma_start(psel_sb[:], psel_d[:])
            nc.gpsimd.dma_start(tmsk_sb[:], tmsk_d[:])

            # ones columns of V_aug; indicator matrix for recip expand
            nc.gpsimd.memset(v_sb[:, :, 64:65], 1.0)
            nc.gpsimd.memset(v_sb[:, :, 129:130], 1.0)
            nc.gpsimd.memset(e_sb[:], 1.0)
            nc.gpsimd.affine_select(
                out=e_sb[:].rearrange("p (s j) -> p s j", s=8),
                in_=e_sb[:].rearrange("p (s j) -> p s j", s=8),
                compare_op=mybir.AluOpType.is_equal,
                fill=0.0, base=0,
                pattern=[[-1, 8], [0, 64]], channel_multiplier=1,
            )

            # preload the exp table set
            warm = wpool.tile([128, 1], F32)
            nc.gpsimd.memset(warm[:], 0.0)
            nc.scalar.activation(warm[:], warm[:], Exp)

            def load_wo(part):  # part in 0..1, 8 k-tiles each
                nc.gpsimd.dma_start(
                    wo_sb[:, 8 * part:8 * (part + 1), :],
                    wo_d[DIM // 2 * part: DIM // 2 * (part + 1), :]
                    .rearrange("(t p) c -> p t c", p=128))

            # goffc register (per-core gather column offset)
            gc_reg = nc.sync.alloc_register("gc_reg")
            nc.sync.reg_load(gc_reg, goffc_d[0:1, 0:1])
            goffc = nc.sync.snap(gc_reg, donate=True, min_val=0,
                                 max_val=CHUNK - 128)

            # ---------------- helpers ----------------
            xT_tiles = {}

            def transposes(c):
                xT = io.tile([128, DT, CHUNK], BF16, tag="xT", bufs=2,
                             name=f"xT{c}")
                xT_tiles[c] = xT
                # all transposes on ONE ring: concurrent transposes from
                # both HWDGE rings corrupt data (shared XBAR)
                for dt in range(DT):
                    nc.sync.dma_start_transpose(
                        xT[:, dt, :],
                        xbf[c * CHUNK:(c + 1) * CHUNK,
                            dt * 128:(dt + 1) * 128])

            qt_tiles = {}

            def rope(raw_ps, c, out_ap, nheads2):
                """raw_ps: [128, 512] psum with 2 stacked heads (transposed).
                Produces roped bf16 into out_ap."""
                raw = io.tile([128, CHUNK], BF16, tag="qraw", bufs=2,
                              name="qraw")
                nc.vector.tensor_copy(raw[:], raw_ps[:])
                sw_ps = accps.tile([128, CHUNK], F32, tag="acc", bufs=2,
                                   name="swps")
                nc.tensor.matmul(sw_ps[:], psw_sb[:], raw[:],
                                 start=True, stop=True)
                t1 = io.tile([128, CHUNK], BF16, tag="t1", bufs=2, name="t1")
                t2 = io.tile([128, CHUNK], BF16, tag="t2", bufs=2, name="t2")
                cs = cst_sb[:, c * CHUNK:(c + 1) * CHUNK]
                sn = snt_sb[:, c * CHUNK:(c + 1) * CHUNK]
                nc.vector.tensor_mul(t1[:], raw[:], cs)
                nc.vector.tensor_mul(t2[:], sw_ps[:], sn)
                nc.vector.tensor_add(out_ap, t1[:], t2[:])

            def proj(c):
                xT = xT_tiles[c]
                qt = io.tile([128, 4, CHUNK], BF16, tag="qt", bufs=2,
                             name=f"qt{c}")
                qt_tiles[c] = qt
                for sp in range(4):
                    q_ps = accps.tile([128, CHUNK], F32, tag="acc", bufs=2,
                                      name="qps")
                    for dt in range(DT):
                        nc.tensor.matmul(q_ps[:], wq_sb[:, dt, 128 * sp:
                                                        128 * (sp + 1)],
                                         xT[:, dt, :],
                                         start=(dt == 0), stop=(dt == DT - 1))
                    rope(q_ps, c, qt[:, sp, :], 2)
                k_ps = accps.tile([128, CHUNK], F32, tag="acc", bufs=2,
                                  name="kps")
                for dt in range(DT):
                    nc.tensor.matmul(k_ps[:], wk_sb[:, dt, :], xT[:, dt, :],
                                     start=(dt == 0), stop=(dt == DT - 1))
                rope(k_ps, c, kt_sb[:, c * CHUNK:(c + 1) * CHUNK], 2)
                # V natural: xT stationary, wv moving
                v_ps = accps.tile([128, 4, 128], F32, tag="acc", bufs=2,
                                  name="vps")
                # sub-major: one accumulation group at a time — start=True
                # clears has_written for the WHOLE bank, so interleaving
                # groups within one bank drops contributions
                for sub in range(4):
                    for dt in range(DT):
                        nc.tensor.matmul(
                            v_ps[:, sub, :],
                            xT[:, dt, sub * 128:(sub + 1) * 128],
                            wv_sb[:, dt, :],
                            start=(dt == 0), stop=(dt == DT - 1))
                for sub in range(4):
                    gt = 4 * c + sub
                    nc.vector.tensor_copy(v_sb[:, gt, 0:64],
                                          v_ps[:, sub, 0:64])
                    nc.vector.tensor_copy(v_sb[:, gt, 65:129],
                                          v_ps[:, sub, 64:128])

            stages = {}       # c -> list of 8 stage tiles [65, 512]
            recips = {}       # c -> recip8b tile
            ag_dmas = {}      # c -> staging dmas
            cc_insts = {}     # c -> AG collective inst

            def attention(c):
                stages[c] = []
                den_ps = apsps.tile([128, 512], F32, tag="den", bufs=1,
                                    name="denps")
                for sp in range(4):
                    aps = [apsps.tile([128, 512], F32, tag="attn", bufs=2,
                                      name=f"attn{j}") for j in range(2)]
                    prev = None  # pending PV (pt, kt, vs)
                    for kt in range(4 * c + 4):
                        vs = max(0, 128 * kt - CHUNK * c)
                        spt = [spsps.tile([128, 512], F32, tag="sps",
                                          bufs=3, name="spt")
                               for _j in range(2)]
                        for j in range(2):
                            nc.tensor.matmul(
                                spt[j][:, vs:512],
                                kt_sb[64 * j:64 * j + 64,
                                      kt * 128:(kt + 1) * 128],
                                qt_tiles[c][64 * j:64 * j + 64, sp,
                                            vs:CHUNK],
                                start=True, stop=True)
                        if prev is not None:
                            ppt, pkt, pvs = prev
                            for j in range(2):
                                nc.tensor.matmul(
                                    aps[j][0:65, pvs:512],
                                    v_sb[:, pkt, 65 * j:65 * j + 65],
                                    ppt[j][:, pvs:512],
                                    start=(pkt == 0),
                                    stop=(pkt == 4 * c + 3))
                        pt = [io.tile([128, 512], BF16, tag="pT", bufs=6,
                                      name="pt") for _j in range(2)]
                        for j in range(2):
                            nc.scalar.activation(pt[j][:, vs:512],
                                                 spt[j][:, vs:512],
                                                 Exp, scale=0.125)
                            if kt >= 4 * c:
                                nc.vector.tensor_mul(pt[j][:, vs:vs + 128],
                                                     pt[j][:, vs:vs + 128],
                                                     tmsk_sb[:])
                        prev = (pt, kt, vs)
                    ppt, pkt, pvs = prev
                    for j in range(2):
                        nc.tensor.matmul(
                            aps[j][0:65, pvs:512],
                            v_sb[:, pkt, 65 * j:65 * j + 65],
                            ppt[j][:, pvs:512],
                            start=(pkt == 0), stop=(pkt == 4 * c + 3))
                    for j in range(2):
                        s = 2 * sp + j
                        stg = io.tile([65, 512], BF16, tag="stage", bufs=16,
                                      name="stg")
                        nc.vector.tensor_copy(stg[:], aps[j][0:65, :])
                        stages[c].append(stg)
                        nc.tensor.matmul(den_ps[0:8, :],
                                         psel_sb[0:65, 8 * s:8 * (s + 1)],
                                         stg[:],
                                         start=(s == 0), stop=(s == 7))
                    if sp == 0 and c >= 1:
                        scale_chunk(c - 1)
                        emit_ag(c - 1)
                recip8 = io.tile([8, 512], F32, tag="recip", bufs=1,
                                 name="recip8")
                nc.vector.reciprocal(recip8[:], den_ps[0:8, :])
                recip8b = io.tile([8, 512], BF16, tag="recipb", bufs=2,
                                  name="recip8b")
                nc.vector.tensor_copy(recip8b[:], recip8[:])
                recips[c] = recip8b

            def scale_chunk(c):
                ag_dmas[c] = []
                for s in range(8):
                    rexp = accps.tile([128, 512], F32, tag="acc", bufs=2,
                                      name="rexp")
                    nc.tensor.matmul(rexp[0:64, :],
                                     e_sb[:, 64 * s:64 * s + 64],
                                     recips[c][:], start=True, stop=True)
                    sts = io.tile([64, 512], BF16, tag="stS", bufs=2,
                                  name="sts")
                    nc.vector.tensor_mul(sts[:], stages[c][s][0:64, :],
                                         rexp[0:64, :])
                    ag_dmas[c].append(nc.scalar.dma_start(
                        agin[c][64 * s:64 * (s + 1), :], sts[:]))

            def emit_ag(c):
                cc = nc.gpsimd.collective_compute(
                    "AllGather", mybir.AluOpType.bypass,
                    replica_groups=GROUPS,
                    ins=[agin[c][:, :].opt()], outs=[gath[c][:, :].opt()])
                for d in ag_dmas[c]:
                    add_dep_helper(cc.ins, d.ins, sync=True,
                                   reason="AG waits its staging DMAs")
                cc_insts[c] = cc

            def outproj(c):
                ag_sb = io.tile([128, DT, 128], BF16, tag="agsb", bufs=1,
                                name="agsb")
                for dt in range(DT):
                    d = nc.sync.dma_start(
                        ag_sb[:, dt, :],
                        gath[c][128 * dt:128 * (dt + 1),
                                bass.ds(goffc, 128)])
                    add_dep_helper(d.ins, cc_insts[c].ins, sync=True,
                                   reason="gather read waits its AG")
                for half in range(2):
                    ops = [accps.tile([128, 512], F32, tag="acc", bufs=2,
                                      name=f"wops{i}") for i in range(2)]
                    for dt in range(DT):
                        for nb in range(2):
                            col = 1024 * half + 512 * nb
                            nc.tensor.matmul(
                                ops[nb][:], ag_sb[:, dt, :],
                                wo_sb[:, dt, col:col + 512],
                                start=(dt == 0), stop=(dt == DT - 1))
                    for nb in range(2):
                        col = 1024 * half + 512 * nb
                        outs = io.tile([128, 512], F32, tag="outs", bufs=2,
                                       name="outs")
                        nc.vector.tensor_copy(outs[:], ops[nb][:])
                        nc.sync.dma_start(
                            out_d[c * 128:(c + 1) * 128, col:col + 512],
                            outs[:])

            # ---------------- schedule ----------------
            transposes(0)
            load_wo(0)
            stage_x(1)
            transposes(1)
            proj(0)
            load_wo(1)
            for i in range(1, 5):
                attention(i - 1)       # runs scale(i-2)+AG(i-2) at sp0
                if i <= 3:
                    proj(i)
                if i <= 2:
                    stage_x(i + 1)
                    transposes(i + 1)
                if i >= 2:
                    outproj(i - 2)
            scale_chunk(3)
            emit_ag(3)
            outproj(3)

    nc.finalize()
    return nc


_NC_CACHE = None


def _get_nc():
    global _NC_CACHE
    if _NC_CACHE is None:
        _NC_CACHE = _build()
    return _NC_CACHE


def _shard_inputs(x, wq, wk, wv, wo, freqs_cos, freqs_sin):
    """Pure layout work: slice batch, pick each core's heads, permute rope
    pairs within each head, permute wo rows to match the slot order, and
    lay out the rope cos/sin tables for the transposed [hd, seq] domain."""
    x = np.ascontiguousarray(np.asarray(x, dtype=np.float32))
    wq = np.asarray(wq, dtype=np.float32)
    wk = np.asarray(wk, dtype=np.float32)
    wv = np.asarray(wv, dtype=np.float32)
    wo = np.asarray(wo, dtype=np.float32)
    cos = np.asarray(freqs_cos, dtype=np.float32)
    sin = np.asarray(freqs_sin, dtype=np.float32)

    # transposed-layout rope tables: per 64-row head block, rows 0-31 hold
    # the "even" half, rows 32-63 the "odd" half (HD_PERM order).
    cos_t = cos.T                          # [32, S]
    sin_t = sin.T
    cst = np.ascontiguousarray(np.tile(cos_t, (4, 1)))           # [128, S]
    snt = np.ascontiguousarray(
        np.concatenate([-sin_t, sin_t, -sin_t, sin_t], axis=0))  # [128, S]

    # block swap permutation (symmetric): row i <- row (i +- 32) within
    # each 64-row block
    psw = np.zeros((128, 128), dtype=np.float32)
    for i in range(128):
        j = (i // 64) * 64 + (i % 64 + 32) % 64
        psw[i, j] = 1.0

    # causal mask for diagonal 128-tiles: keep query-col >= key-row
    tmsk = np.tril(np.ones((128, 128), dtype=np.float32)).T
    tmsk = np.ascontiguousarray(tmsk)

    # denominator gather: matmul s uses cols [8s:8s+8); row 64 of col
    # (8s+s) is 1 so out row s accumulates the denominator of slot s.
    psel = np.zeros((128, 64), dtype=np.float32)
    for s in range(8):
        psel[64, 8 * s + s] = 1.0

    # wo rows permuted once: gathered row 512*g + 64*s + d  <-  head
    # 8g + slot(s)
    wo_perm = np.empty_like(wo)
    for g in range(4):
        for s_ in range(8):
            h = 8 * g + SLOT_TO_LOCAL[s_]
            wo_perm[512 * g + 64 * s_: 512 * g + 64 * (s_ + 1), :] = \
                wo[64 * h: 64 * (h + 1), :]
    wo_perm = np.ascontiguousarray(wo_perm)

    in_maps = []
    for core in range(N_CORES):
        b, g = core // 4, core % 4
        wq_cols = []
        for s_ in range(8):
            h = 8 * g + SLOT_TO_LOCAL[s_]
            wq_cols.append(wq[:, 64 * h + HD_PERM])
        wq_s = np.ascontiguousarray(np.concatenate(wq_cols, axis=1))
        wk_s = np.ascontiguousarray(np.concatenate(
            [wk[:, 64 * (2 * g + j) + HD_PERM] for j in range(2)], axis=1))
        wv_s = np.ascontiguousarray(wv[:, 128 * g: 128 * (g + 1)])
        in_maps.append({
            "x": x[b], "wq": wq_s, "wk": wk_s, "wv": wv_s, "wo": wo_perm,
            "cst": cst, "snt": snt, "psw": psw, "psel": psel,
            "tmsk": tmsk,
            "goffc": np.array([[128 * g]], dtype=np.uint32),
        })
    return in_maps


def kernel(x, wq, wk, wv, wo, freqs_cos, freqs_sin, mask=None, start_pos=0,
           **_unused):
    nc = _get_nc()
    in_maps = _shard_inputs(x, wq, wk, wv, wo, freqs_cos, freqs_sin)
    res = bass_utils.run_bass_kernel_spmd(
        nc, in_maps, core_ids=list(range(N_CORES)))
    out = np.empty((B, S, DIM), dtype=np.float32)
    for core in range(N_CORES):
        b, g = core // 4, core % 4
        r = res.results[core]["out"]            # [512, DIM]: 4 chunks x 128
        for c in range(NCH):
            out[b, c * CHUNK + 128 * g: c * CHUNK + 128 * (g + 1), :] = \
                r[128 * c: 128 * (c + 1), :]
    return out


# revision 13
# speedup vs baseline: 1.0532x; 1.0532x over previous
"""GQA attention (B=2,S=2048,DIM=2048,H=32,KVH=8,HD=64) + RoPE, causal.

Distributed over 8 TRN2 NeuronCores: core = 4*batch + head_group.
Each core computes attention for its 8 q-heads (2 kv-heads) of one batch.

v2 design (single interleaved per-chunk pipeline):
  - x is staged f32->bf16 by one casting DRAM->DRAM DMA per chunk (gpsimd
    SWDGE), then xbar-transposed into SBUF across both HWDGE rings.
  - Q^T/K^T are produced directly by the PE (weights stationary, x^T
    moving), so no DRAM round-trip for roped q/k.  RoPE runs in the
    transposed [hd, seq] layout: a constant 128x128 block-swap matmul
    provides the rotated halves, then 3 DVE ops combine with cos/sin
    tables (host-prepared constant layout).
  - V is computed naturally (x^T stationary, wv moving) for the PV matmul.
  - Attention per chunk interleaves with the next chunk's projections and
    the previous chunk's output projection in one PE stream.
  - AllGather runs per chunk (4 small collectives), each core then
    output-projects 128 rows per chunk against the full permuted wo.
"""
import numpy as np

import concourse.bass as bass
import concourse.bacc as bacc
import concourse.tile as tile
from concourse.tile import add_dep_helper
import concourse.mybir as mybir
from concourse import bass_utils


def _ensure_axon_hooks_shim():
    """bass_utils imports antenv.axon_hooks when BASS_TRACE is set; the
    module is absent in some images. Provide a no-op shim so tracing env
    vars cannot crash the run."""
    import sys, types
    try:
        import antenv  # noqa
        if "antenv.axon_hooks" in sys.modules:
            return
        import importlib
        try:
            importlib.import_module("antenv.axon_hooks")
            return
        except ImportError:
            pass
        mod = types.ModuleType("antenv.axon_hooks")
        mod._hook = None
        mod.get_axon_ntff_profile_hook = lambda: mod._hook

        def set_axon_ntff_profile_hook(h):
            mod._hook = h
        mod.set_axon_ntff_profile_hook = set_axon_ntff_profile_hook
        sys.modules["antenv.axon_hooks"] = mod
        antenv.axon_hooks = mod
    except Exception:
        pass


_ensure_axon_hooks_shim()

F32 = mybir.dt.float32
BF16 = mybir.dt.bfloat16

B, S, DIM = 2, 2048, 2048
H, KVH, HD = 32, 8, 64
N_CORES = 8
GROUPS = [[0, 1, 2, 3], [4, 5, 6, 7]]
NCH = 4            # sequence chunks of 512
CHUNK = S // NCH   # 512
DT = DIM // 128    # 16 contraction tiles
# q-head slot order: slot s holds local q-head s//2 + 4*(s%2); slot parity
# is the local kv-head. slot-pair sp covers slots (2sp, 2sp+1).
SLOT_TO_LOCAL = [s // 2 + 4 * (s % 2) for s in range(8)]
# rope pair permutation within one head: evens then odds
HD_PERM = np.concatenate([np.arange(0, HD, 2), np.arange(1, HD, 2)])


def _build():
    nc = bacc.Bacc("TRN2", target_bir_lowering=False, debug=False,
                   num_devices=N_CORES)
    x_d = nc.dram_tensor("x", [S, DIM], F32, kind="ExternalInput")
    wq_d = nc.dram_tensor("wq", [DIM, 512], F32, kind="ExternalInput")
    wk_d = nc.dram_tensor("wk", [DIM, 128], F32, kind="ExternalInput")
    wv_d = nc.dram_tensor("wv", [DIM, 128], F32, kind="ExternalInput")
    wo_d = nc.dram_tensor("wo", [DIM, DIM], F32, kind="ExternalInput")
    cst_d = nc.dram_tensor("cst", [128, S], F32, kind="ExternalInput")
    snt_d = nc.dram_tensor("snt", [128, S], F32, kind="ExternalInput")
    psw_d = nc.dram_tensor("psw", [128, 128], F32, kind="ExternalInput")
    psel_d = nc.dram_tensor("psel", [128, 64], F32, kind="ExternalInput")
    tmsk_d = nc.dram_tensor("tmsk", [128, 128], F32, kind="ExternalInput")
    goffc_d = nc.dram_tensor("goffc", [1, 1], mybir.dt.uint32,
                             kind="ExternalInput")
    out_d = nc.dram_tensor("out", [CHUNK, DIM], F32, kind="ExternalOutput")

    Exp = mybir.ActivationFunctionType.Exp

    with tile.TileContext(nc) as tc:
        with tc.tile_pool(name="dram", bufs=1, space="DRAM") as dram, \
             tc.tile_pool(name="wpool", bufs=1) as wpool, \
             tc.tile_pool(name="io", bufs=2) as io, \
             tc.tile_pool(name="accps", bufs=2, space="PSUM") as accps, \
             tc.tile_pool(name="spsps", bufs=2, space="PSUM") as spsps, \
             tc.tile_pool(name="apsps", bufs=2, space="PSUM") as apsps:
            # ---- DRAM scratch ----
            xbf = dram.tile([S, DIM], BF16)
            agin = [dram.tile([512, CHUNK], BF16, name=f"agin{c}")
                    for c in range(NCH)]
            gath = [dram.tile([4 * 512, CHUNK], BF16, name=f"gath{c}")
                    for c in range(NCH)]

            # ---- persistent SBUF ----
            wq_sb = wpool.tile([128, DT, 512], BF16)
            wk_sb = wpool.tile([128, DT, 128], BF16)
            wv_sb = wpool.tile([128, DT, 128], BF16)
            wo_sb = wpool.tile([128, DT, DIM], BF16)
            cst_sb = wpool.tile([128, S], BF16)
            snt_sb = wpool.tile([128, S], BF16)
            psw_sb = wpool.tile([128, 128], BF16)
            psel_sb = wpool.tile([128, 64], BF16)
            tmsk_sb = wpool.tile([128, 128], BF16)
            kt_sb = wpool.tile([128, S], BF16)          # K^T (kv0|kv1)
            v_sb = wpool.tile([128, DT, 130], BF16)     # [V0|1|V1|1] per tile
            e_sb = wpool.tile([8, 512], BF16)           # recip expand indic

            # ---- prologue: stage x chunk 0 first ----
            def stage_x(c):
                # HWDGE f32 load -> DVE cast -> HWDGE store to xbf; keeps
                # the big x traffic off the single SWDGE queue
                for t in range(4):
                    r0 = c * CHUNK + t * 128
                    for hh in range(2):
                        c0 = hh * 1024
                        xf = io.tile([128, 1024], F32, tag="xf", bufs=4,
                                     name="xf")
                        nc.scalar.dma_start(xf[:],
                                            x_d[r0:r0 + 128, c0:c0 + 1024])
                        xb = io.tile([128, 1024], BF16, tag="xb", bufs=2,
                                     name="xb")
                        nc.vector.tensor_copy(xb[:], xf[:])
                        nc.sync.dma_start(xbf[r0:r0 + 128, c0:c0 + 1024],
                                          xb[:])

            stage_x(0)

            # weights + constants (casting SWDGE loads), ordered by need
            nc.gpsimd.dma_start(
                wq_sb[:], wq_d[:].rearrange("(t p) c -> p t c", p=128))
            nc.gpsimd.dma_start(
                wk_sb[:], wk_d[:].rearrange("(t p) c -> p t c", p=128))
            nc.gpsimd.dma_start(
                wv_sb[:], wv_d[:].rearrange("(t p) c -> p t c", p=128))
            nc.gpsimd.dma_start(cst_sb[:], cst_d[:])
            nc.gpsimd.dma_start(snt_sb[:], snt_d[:])
            nc.gpsimd.dma_start(psw_sb[:], psw_d[:])
            nc.gpsimd.dma_start(psel_sb[:], psel_d[:])
            nc.gpsimd.dma_start(tmsk_sb[:], tmsk_d[:])

            # ones columns of V_aug; indicator matrix for recip expand
            nc.gpsimd.memset(v_sb[:, :, 64:65], 1.0)
            nc.gpsimd.memset(v_sb[:, :, 129:130], 1.0)
            nc.gpsimd.memset(e_sb[:], 1.0)
            nc.gpsimd.affine_select(
                out=e_sb[:].rearrange("p (s j) -> p s j", s=8),
                in_=e_sb[:].rearrange("p (s j) -> p s j", s=8),
                compare_op=mybir.AluOpType.is_equal,
                fill=0.0, base=0,
                pattern=[[-1, 8], [0, 64]], channel_multiplier=1,
            )

            # preload the exp table set
            warm = wpool.tile([128, 1], F32)
            nc.gpsimd.memset(warm[:], 0.0)
            nc.scalar.activation(warm[:], warm[:], Exp)

            def load_wo(part):  # part in 0..1, 8 k-tiles each
                nc.gpsimd.dma_start(
                    wo_sb[:, 8 * part:8 * (part + 1), :],
                    wo_d[DIM // 2 * part: DIM // 2 * (part + 1), :]
                    .rearrange("(t p) c -> p t c", p=128))

            # goffc register (per-core gather column offset)
            gc_reg = nc.sync.alloc_register("gc_reg")
            nc.sync.reg_load(gc_reg, goffc_d[0:1, 0:1])
            goffc = nc.sync.snap(gc_reg, donate=True, min_val=0,
                                 max_val=CHUNK - 128)

            # ---------------- helpers ----------------
            xT_tiles = {}

            def transposes(c):
                xT = io.tile([128, DT, CHUNK], BF16, tag="xT", bufs=2,
                             name=f"xT{c}")
                xT_tiles[c] = xT
                # all transposes on ONE ring: concurrent transposes from
                # both HWDGE rings corrupt data (shared XBAR)
                for dt in range(DT):
                    nc.sync.dma_start_transpose(
                        xT[:, dt, :],
                        xbf[c * CHUNK:(c + 1) * CHUNK,
                            dt * 128:(dt + 1) * 128])

            qt_tiles = {}

            def rope(raw_ps, c, out_ap, nheads2):
                """raw_ps: [128, 512] psum with 2 stacked heads (transposed).
                Produces roped bf16 into out_ap."""
                raw = io.tile([128, CHUNK], BF16, tag="qraw", bufs=2,
                              name="qraw")
                nc.vector.tensor_copy(raw[:], raw_ps[:])
                sw_ps = accps.tile([128, CHUNK], F32, tag="acc", bufs=2,
                                   name="swps")
                nc.tensor.matmul(sw_ps[:], psw_sb[:], raw[:],
                                 start=True, stop=True)
                t1 = io.tile([128, CHUNK], BF16, tag="t1", bufs=2, name="t1")
                t2 = io.tile([128, CHUNK], BF16, tag="t2", bufs=2, name="t2")
                cs = cst_sb[:, c * CHUNK:(c + 1) * CHUNK]
                sn = snt_sb[:, c * CHUNK:(c + 1) * CHUNK]
                nc.vector.tensor_mul(t1[:], raw[:], cs)
                nc.vector.tensor_mul(t2[:], sw_ps[:], sn)
                nc.vector.tensor_add(out_ap, t1[:], t2[:])

            def proj(c):
                xT = xT_tiles[c]
                qt = io.tile([128, 4, CHUNK], BF16, tag="qt", bufs=2,
                             name=f"qt{c}")
                qt_tiles[c] = qt
                for sp in range(4):
                    q_ps = accps.tile([128, CHUNK], F32, tag="acc", bufs=2,
                                      name="qps")
                    for dt in range(DT):
                        nc.tensor.matmul(q_ps[:], wq_sb[:, dt, 128 * sp:
                                                        128 * (sp + 1)],
                                         xT[:, dt, :],
                                         start=(dt == 0), stop=(dt == DT - 1))
                    rope(q_ps, c, qt[:, sp, :], 2)
                k_ps = accps.tile([128, CHUNK], F32, tag="acc", bufs=2,
                                  name="kps")
                for dt in range(DT):
                    nc.tensor.matmul(k_ps[:], wk_sb[:, dt, :], xT[:, dt, :],
                                     start=(dt == 0), stop=(dt == DT - 1))
                rope(k_ps, c, kt_sb[:, c * CHUNK:(c + 1) * CHUNK], 2)
                # V natural: xT stationary, wv moving
                v_ps = accps.tile([128, 4, 128], F32, tag="acc", bufs=2,
                                  name="vps")
                # sub-major: one accumulation group at a time — start=True
                # clears has_written for the WHOLE bank, so interleaving
                # groups within one bank drops contributions
                for sub in range(4):
                    for dt in range(DT):
                        nc.tensor.matmul(
                            v_ps[:, sub, :],
                            xT[:, dt, sub * 128:(sub + 1) * 128],
                            wv_sb[:, dt, :],
                            start=(dt == 0), stop=(dt == DT - 1))
                for sub in range(4):
                    gt = 4 * c + sub
                    nc.vector.tensor_copy(v_sb[:, gt, 0:64],
                                          v_ps[:, sub, 0:64])
                    nc.vector.tensor_copy(v_sb[:, gt, 65:129],
                                          v_ps[:, sub, 64:128])

            stages = {}       # c -> list of 8 stage tiles [65, 512]
            recips = {}       # c -> recip8b tile
            ag_dmas = {}      # c -> staging dmas
            cc_insts = {}     # c -> AG collective inst

            def attention(c):
                stages[c] = []
                den_ps = apsps.tile([128, 512], F32, tag="den", bufs=1,
                                    name="denps")
                for sp in range(4):
                    aps = [apsps.tile([128, 512], F32, tag="attn", bufs=2,
                                      name=f"attn{j}") for j in range(2)]
                    prev = None  # pending PV (pt, kt, vs)
                    for kt in range(4 * c + 4):
                        vs = max(0, 128 * kt - CHUNK * c)
                        spt = [spsps.tile([128, 512], F32, tag="sps",
                                          bufs=3, name="spt")
                               for _j in range(2)]
                        for j in range(2):
                            nc.tensor.matmul(
                                spt[j][:, vs:512],
                                kt_sb[64 * j:64 * j + 64,
                                      kt * 128:(kt + 1) * 128],
                                qt_tiles[c][64 * j:64 * j + 64, sp,
                                            vs:CHUNK],
                                start=True, stop=True)
                        if prev is not None:
                            ppt, pkt, pvs = prev
                            for j in range(2):
                                nc.tensor.matmul(
                                    aps[j][0:65, pvs:512],
                                    v_sb[:, pkt, 65 * j:65 * j + 65],
                                    ppt[j][:, pvs:512],
                                    start=(pkt == 0),
                                    stop=(pkt == 4 * c + 3))
                        pt = [io.tile([128, 512], BF16, tag="pT", bufs=6,
                                      name="pt") for _j in range(2)]
                        for j in range(2):
                            nc.scalar.activation(pt[j][:, vs:512],
                                                 spt[j][:, vs:512],
                                                 Exp, scale=0.125)
                            if kt >= 4 * c:
                                nc.vector.tensor_mul(pt[j][:, vs:vs + 128],
                                                     pt[j][:, vs:vs + 128],
                                                     tmsk_sb[:])
                        prev = (pt, kt, vs)
                    ppt, pkt, pvs = prev
                    for j in range(2):
                        nc.tensor.matmul(
                            aps[j][0:65, pvs:512],
                            v_sb[:, pkt, 65 * j:65 * j + 65],
                            ppt[j][:, pvs:512],
                            start=(pkt == 0), stop=(pkt == 4 * c + 3))
                    for j in range(2):
                        s = 2 * sp + j
                        stg = io.tile([65, 512], BF16, tag="stage", bufs=16,
                                      name="stg")
                        nc.vector.tensor_copy(stg[:], aps[j][0:65, :])
                        stages[c].append(stg)
                        nc.tensor.matmul(den_ps[0:8, :],
                                         psel_sb[0:65, 8 * s:8 * (s + 1)],
                                         stg[:],
                                         start=(s == 0), stop=(s == 7))
                    if sp == 0 and c >= 1:
                        scale_chunk(c - 1)
                        emit_ag(c - 1)
                recip8 = io.tile([8, 512], F32, tag="recip", bufs=1,
                                 name="recip8")
                nc.vector.reciprocal(recip8[:], den_ps[0:8, :])
                recip8b = io.tile([8, 512], BF16, tag="recipb", bufs=2,
                                  name="recip8b")
                nc.vector.tensor_copy(recip8b[:], recip8[:])
                recips[c] = recip8b

            def scale_chunk(c):
                ag_dmas[c] = []
                for s in range(8):
                    rexp = accps.tile([128, 512], F32, tag="acc", bufs=2,
                                      name="rexp")
                    nc.tensor.matmul(rexp[0:64, :],
                                     e_sb[:, 64 * s:64 * s + 64],
                                     recips[c][:], start=True, stop=True)
                    sts = io.tile([64, 512], BF16, tag="stS", bufs=2,
                                  name="sts")
                    nc.vector.tensor_mul(sts[:], stages[c][s][0:64, :],
                                         rexp[0:64, :])
                    ag_dmas[c].append(nc.scalar.dma_start(
                        agin[c][64 * s:64 * (s + 1), :], sts[:]))

            def emit_ag(c):
                cc = nc.gpsimd.collective_compute(
                    "AllGather", mybir.AluOpType.bypass,
                    replica_groups=GROUPS,
                    ins=[agin[c][:, :].opt()], outs=[gath[c][:, :].opt()])
                for d in ag_dmas[c]:
                    add_dep_helper(cc.ins, d.ins, sync=True,
                                   reason="AG waits its staging DMAs")
                cc_insts[c] = cc

            def outproj(c):
                ag_sb = io.tile([128, DT, 128], BF16, tag="agsb", bufs=1,
                                name="agsb")
                for dt in range(DT):
                    d = nc.sync.dma_start(
                        ag_sb[:, dt, :],
                        gath[c][128 * dt:128 * (dt + 1),
                                bass.ds(goffc, 128)])
                    add_dep_helper(d.ins, cc_insts[c].ins, sync=True,
                                   reason="gather read waits its AG")
                for half in range(2):
                    ops = [accps.tile([128, 512], F32, tag="acc", bufs=2,
                                      name=f"wops{i}") for i in range(2)]
                    for dt in range(DT):
                        for nb in range(2):
                            col = 1024 * half + 512 * nb
                            nc.tensor.matmul(
                                ops[nb][:], ag_sb[:, dt, :],
                                wo_sb[:, dt, col:col + 512],
                                start=(dt == 0), stop=(dt == DT - 1))
                    for nb in range(2):
                        col = 1024 * half + 512 * nb
                        outs = io.tile([128, 512], F32, tag="outs", bufs=2,
                                       name="outs")
                        nc.vector.tensor_copy(outs[:], ops[nb][:])
                        nc.sync.dma_start(
                            out_d[c * 128:(c + 1) * 128, col:col + 512],
                            outs[:])

            # ---------------- schedule ----------------
            transposes(0)
            load_wo(0)
            stage_x(1)
            transposes(1)
            proj(0)
            load_wo(1)
            for i in range(1, 5):
                if i <= 2:
                    stage_x(i + 1)
                    transposes(i + 1)
                attention(i - 1)       # runs scale(i-2)+AG(i-2) at sp0
                if i <= 3:
                    proj(i)
                if i >= 2:
                    outproj(i - 2)
            scale_chunk(3)
            emit_ag(3)
            outproj(3)

    nc.finalize()
    return nc


_NC_CACHE = None


def _get_nc():
    global _NC_CACHE
    if _NC_CACHE is None:
        _NC_CACHE = _build()
    return _NC_CACHE


def _shard_inputs(x, wq, wk, wv, wo, freqs_cos, freqs_sin):
    """Pure layout work: slice batch, pick each core's heads, permute rope
    pairs within each head, permute wo rows to match the slot order, and
    lay out the rope cos/sin tables for the transposed [hd, seq] domain."""
    x = np.ascontiguousarray(np.asarray(x, dtype=np.float32))
    wq = np.asarray(wq, dtype=np.float32)
    wk = np.asarray(wk, dtype=np.float32)
    wv = np.asarray(wv, dtype=np.float32)
    wo = np.asarray(wo, dtype=np.float32)
    cos = np.asarray(freqs_cos, dtype=np.float32)
    sin = np.asarray(freqs_sin, dtype=np.float32)

    # transposed-layout rope tables: per 64-row head block, rows 0-31 hold
    # the "even" half, rows 32-63 the "odd" half (HD_PERM order).
    cos_t = cos.T                          # [32, S]
    sin_t = sin.T
    cst = np.ascontiguousarray(np.tile(cos_t, (4, 1)))           # [128, S]
    snt = np.ascontiguousarray(
        np.concatenate([-sin_t, sin_t, -sin_t, sin_t], axis=0))  # [128, S]

    # block swap permutation (symmetric): row i <- row (i +- 32) within
    # each 64-row block
    psw = np.zeros((128, 128), dtype=np.float32)
    for i in range(128):
        j = (i // 64) * 64 + (i % 64 + 32) % 64
        psw[i, j] = 1.0

    # causal mask for diagonal 128-tiles: keep query-col >= key-row
    tmsk = np.tril(np.ones((128, 128), dtype=np.float32)).T
    tmsk = np.ascontiguousarray(tmsk)

    # denominator gather: matmul s uses cols [8s:8s+8); row 64 of col
    # (8s+s) is 1 so out row s accumulates the denominator of slot s.
    psel = np.zeros((128, 64), dtype=np.float32)
    for s in range(8):
        psel[64, 8 * s + s] = 1.0

    # wo rows permuted once: gathered row 512*g + 64*s + d  <-  head
    # 8g + slot(s)
    wo_perm = np.empty_like(wo)
    for g in range(4):
        for s_ in range(8):
            h = 8 * g + SLOT_TO_LOCAL[s_]
            wo_perm[512 * g + 64 * s_: 512 * g + 64 * (s_ + 1), :] = \
                wo[64 * h: 64 * (h + 1), :]
    wo_perm = np.ascontiguousarray(wo_perm)

    in_maps = []
    for core in range(N_CORES):
        b, g = core // 4, core % 4
        wq_cols = []
        for s_ in range(8):
            h = 8 * g + SLOT_TO_LOCAL[s_]
            wq_cols.append(wq[:, 64 * h + HD_PERM])
        wq_s = np.ascontiguousarray(np.concatenate(wq_cols, axis=1))
        wk_s = np.ascontiguousarray(np.concatenate(
            [wk[:, 64 * (2 * g + j) + HD_PERM] for j in range(2)], axis=1))
        wv_s = np.ascontiguousarray(wv[:, 128 * g: 128 * (g + 1)])
        in_maps.append({
            "x": x[b], "wq": wq_s, "wk": wk_s, "wv": wv_s, "wo": wo_perm,
            "cst": cst, "snt": snt, "psw": psw, "psel": psel,
            "tmsk": tmsk,
            "goffc": np.array([[128 * g]], dtype=np.uint32),
        })
    return in_maps


def kernel(x, wq, wk, wv, wo, freqs_cos, freqs_sin, mask=None, start_pos=0,
           **_unused):
    nc = _get_nc()
    in_maps = _shard_inputs(x, wq, wk, wv, wo, freqs_cos, freqs_sin)
    res = bass_utils.run_bass_kernel_spmd(
        nc, in_maps, core_ids=list(range(N_CORES)))
    out = np.empty((B, S, DIM), dtype=np.float32)
    for core in range(N_CORES):
        b, g = core // 4, core % 4
        r = res.results[core]["out"]            # [512, DIM]: 4 chunks x 128
        for c in range(NCH):
            out[b, c * CHUNK + 128 * g: c * CHUNK + 128 * (g + 1), :] = \
                r[128 * c: 128 * (c + 1), :]
    return out
